# revision 15
# baseline (speedup 1.0000x reference)
"""BitSelfAttention (relative_key_query position bias) on 8 trn2 cores.

Sharding: core c -> batch b=c//2, head-group g=c%2 (8 heads of 64 dims).
Per core: q/k/v projections for its 512 output dims, then per-head
attention with the Toeplitz relative-position bias realized via a DRAM
round-trip (skewed access patterns) for the Eq/Ek tables.

v3 structure:
 - Eq/Ek band-table matmuls run in fp8e4 DoubleRow perf mode (2x PE
   throughput).  q/k fp8 operands are built by gpsimd casting DMAs into
   [32, 2(ktile), S] layouts; the dist table is pre-scaled x64 host-side
   (fp8 dynamic range) and the evacuation copy divides by 64.
 - Band sections are EMITTED between projection blocks so the PE stream
   stays dense through the unthrottled startup window (the tensor engine
   executes its queue in program order).
 - Band PSUM is one [128,1152] tile (3 banks) so each table evacuates in
   a single engine op (eq -> scalar Copy/64 -> bf16, ek -> vector mul/64
   -> fp8).
 - No identity-matmul rel injection: a DVE scalar_tensor_tensor adds rel
   to the scores PSUM and writes bf16 to SBUF, freeing the PSUM bank
   before the exp; the attention mask rides the exp as a per-partition
   bias AP.
"""
import math
from contextlib import ExitStack

import numpy as np

import concourse.bass as bass
import concourse.bacc as bacc
import concourse.tile as tile
from concourse import mybir
from concourse.bass_utils import run_bass_kernel_spmd

B, S, D, H = 4, 1024, 1024, 16
HD = 64
E = 512          # output dims per core (8 heads)
NHC = 8          # heads per core
WD = 2176        # scratch DRAM row width (896 + 1280)
F32 = mybir.dt.float32
BF16 = mybir.dt.bfloat16
FP8 = mybir.dt.float8e4
DR = mybir.MatmulPerfMode.DoubleRow
ALU = mybir.AluOpType
ACT = mybir.ActivationFunctionType


def build_nc():
    nc = bacc.Bacc()
    hT = nc.declare_dram_parameter("hT", [D, S], BF16, isOutput=False)
    wqT = nc.declare_dram_parameter("wqT", [D, E], BF16, isOutput=False)
    wkT = nc.declare_dram_parameter("wkT", [D, E], BF16, isOutput=False)
    wvT = nc.declare_dram_parameter("wvT", [D, E], BF16, isOutput=False)
    bqr = nc.declare_dram_parameter("bqr", [1, E], BF16, isOutput=False)
    bkr = nc.declare_dram_parameter("bkr", [1, E], BF16, isOutput=False)
    bvr = nc.declare_dram_parameter("bvr", [1, E], BF16, isOutput=False)
    # fp8 dist tables, x64-scaled, [32, 2(ktile), WD] replicated on the
    # 0/32/64 partition blocks so the q8/k8 lhsT base always matches.
    de8R = nc.declare_dram_parameter("de8R", [128, 2 * WD], FP8, isOutput=False)
    de8P = nc.declare_dram_parameter("de8P", [128, 2 * WD], FP8, isOutput=False)
    mcol = nc.declare_dram_parameter("mcol", [128, 8], F32, isOutput=False)
    ident = nc.declare_dram_parameter("ident", [128, 128], FP8, isOutput=False)
    out_t = nc.declare_dram_parameter("out", [S, E], F32, isOutput=True)

    # DRAM scratch, fresh per head (no WAR fan-in on reuse)
    eqr = [nc.dram_tensor(f"eqr{i}", [S, WD], FP8) for i in range(8)]
    ekd = [nc.dram_tensor(f"ekd{i}", [S, WD], FP8) for i in range(8)]

    ctx = ExitStack()
    with ctx:
        tc = ctx.enter_context(tile.TileContext(nc))
        consts = ctx.enter_context(tc.tile_pool(name="consts", bufs=1))
        # PSUM (8 banks): band [128,1024] 2bk x2 + tail 1bk + scores/proj/ctx 1bk x3
        band_ps = ctx.enter_context(tc.tile_pool(name="band_ps", bufs=2, space="PSUM"))
        tail_ps = ctx.enter_context(tc.tile_pool(name="tail_ps", bufs=1, space="PSUM"))
        score_ps = ctx.enter_context(tc.tile_pool(name="score_ps", bufs=3, space="PSUM"))
        band_sb = ctx.enter_context(tc.tile_pool(name="band_sb", bufs=8))
        relq_pool = ctx.enter_context(tc.tile_pool(name="relq_pool", bufs=16))
        relk_pool = ctx.enter_context(tc.tile_pool(name="relk_pool", bufs=16))
        expt_pool = ctx.enter_context(tc.tile_pool(name="expt_pool", bufs=10))
        exin_pool = ctx.enter_context(tc.tile_pool(name="exin_pool", bufs=3))
        small = ctx.enter_context(tc.tile_pool(name="small", bufs=4))

        # ---- load inputs to SBUF (one big DMA per tensor) ----
        ht_big = consts.tile([128, 8, S], BF16, name="ht_big")
        nc.sync.dma_start(
            out=ht_big,
            in_=bass.AP(tensor=hT, offset=0,
                        ap=[[S, 128], [128 * S, 8], [1, S]]))
        ht_sb = [ht_big[:, kt, :] for kt in range(8)]
        wq_sb, wk_sb, wv_sb = [], [], []
        for (dst, src, nm) in ((wq_sb, wqT, "wq"), (wk_sb, wkT, "wk"), (wv_sb, wvT, "wv")):
            big = consts.tile([128, 8, E], BF16, name=f"{nm}_big")
            nc.sync.dma_start(
                out=big,
                in_=bass.AP(tensor=src, offset=0,
                            ap=[[E, 128], [128 * E, 8], [1, E]]))
            for kt in range(8):
                dst.append(big[:, kt, :])
        de8r_sb = consts.tile([128, 2, WD], FP8, name="de8r_sb")
        nc.scalar.dma_start(out=de8r_sb,
                          in_=bass.AP(tensor=de8R, offset=0,
                                      ap=[[2 * WD, 128], [WD, 2], [1, WD]]))
        de8p_sb = consts.tile([128, 2, WD], FP8, name="de8p_sb")
        nc.scalar.dma_start(out=de8p_sb,
                          in_=bass.AP(tensor=de8P, offset=0,
                                      ap=[[2 * WD, 128], [WD, 2], [1, WD]]))
        mcol_sb = consts.tile([128, 8], F32, name="mcol_sb")
        nc.scalar.dma_start(out=mcol_sb, in_=mcol[:, :])
        id_sb = consts.tile([128, 128], FP8, name="id_sb")
        nc.scalar.dma_start(out=id_sb, in_=ident[:, :])
        br_sb = {}
        for nm, src in (("bq", bqr), ("bk", bkr), ("bv", bvr)):
            t = consts.tile([1, E], BF16, name=f"{nm}_sb")
            nc.scalar.dma_start(out=t, in_=src[:, :])
            br_sb[nm] = t
        ones_sb = consts.tile([1, E], BF16, name="ones_sb")
        nc.vector.memset(ones_sb, 1.0)

        qT_sb = [consts.tile([128, S], BF16, name=f"qT{et}") for et in range(4)]
        kT_sb = [consts.tile([128, S], BF16, name=f"kT{et}") for et in range(4)]
        # fp8 [32,2,S] operands, 3 heads per tile (base partition 0/32/64)
        q8t = [consts.tile([96, 2, S], FP8, name=f"q8{x}") for x in "ABC"]
        k8t = [consts.tile([96, 2, S], FP8, name=f"k8{x}") for x in "ABC"]
        v_sb = [consts.tile([128, 8, 65], BF16, name=f"v{st}") for st in range(8)]
        out_sb = consts.tile([128, 8, E], F32, name="out_sb")

        def proj_qk(et):
            for (w_sb, bias, dstl) in ((wq_sb, "bq", qT_sb), (wk_sb, "bk", kT_sb)):
                for ns in range(2):
                    ps = score_ps.tile([128, 512], F32, name="ps_proj", tag="sc")
                    for kt in range(8):
                        nc.tensor.matmul(
                            ps, w_sb[kt][:, et * 128:(et + 1) * 128],
                            ht_sb[kt][:, ns * 512:(ns + 1) * 512],
                            start=(kt == 0), stop=False)
                    nc.tensor.matmul(
                        ps, br_sb[bias][0:1, et * 128:(et + 1) * 128],
                        ones_sb[0:1, 0:512], start=False, stop=True)
                    if ns == 0:
                        nc.vector.tensor_copy(dstl[et][:, 0:512], ps)
                    else:
                        nc.scalar.copy(dstl[et][:, 512:1024], ps)
            # fp8 prep for heads 2*et, 2*et+1 (gpsimd casting DMAs)
            for g in range(2):
                h = 2 * et + g
                X, pb = h // 3, 32 * (h % 3)
                for kt in range(2):
                    nc.gpsimd.dma_start(
                        out=q8t[X][pb:pb + 32, kt, :],
                        in_=qT_sb[et][64 * g + 32 * kt:64 * g + 32 * kt + 32, :])
                    nc.gpsimd.dma_start(
                        out=k8t[X][pb:pb + 32, kt, :],
                        in_=kT_sb[et][64 * g + 32 * kt:64 * g + 32 * kt + 32, :])

        def proj_v(st):
            ps = score_ps.tile([128, 512], F32, name="ps_proj", tag="sc")
            for kt in range(8):
                nc.tensor.matmul(
                    ps, ht_sb[kt][:, st * 128:(st + 1) * 128],
                    wv_sb[kt], start=(kt == 0), stop=False)
            nc.tensor.matmul(ps, ones_sb[0:1, 0:128], br_sb["bv"],
                             start=False, stop=True)
            nc.vector.tensor_copy(v_sb[st][:, :, 0:64],
                                  ps.rearrange("p (h e) -> p h e", h=8))
            nc.vector.memset(v_sb[st][:, :, 64:65], 1.0)

        def bands(h):
            X, pb = h // 3, 32 * (h % 3)
            for i in range(8):
                base = 896 - 128 * i
                lq = q8t[X][pb:pb + 32, :, i * 128:(i + 1) * 128]
                bA = band_ps.tile([128, 1024], F32, name="bA", tag="bA")
                for wo in (0, 512):
                    nc.tensor.matmul(
                        bA[:, wo:wo + 512], lq,
                        de8r_sb[pb:pb + 32, :, base + wo:base + wo + 512],
                        start=True, stop=True, perf_mode=DR)
                bB = tail_ps.tile([128, 128], F32, name="bB", tag="bB")
                nc.tensor.matmul(
                    bB, lq,
                    de8r_sb[pb:pb + 32, :, base + 1024:base + 1152],
                    start=True, stop=True, perf_mode=DR)
                eq_stage = band_sb.tile([128, 1152], FP8, name="eq_stage", tag="eq_stage")
                nc.scalar.activation(out=eq_stage[:, 0:1024], in_=bA,
                                     func=ACT.Copy, scale=1.0 / 64.0)
                nc.scalar.activation(out=eq_stage[:, 1024:1152], in_=bB,
                                     func=ACT.Copy, scale=1.0 / 64.0)
                nc.sync.dma_start(
                    out=bass.AP(tensor=eqr[h],
                                offset=128 * i * WD + 896 - 128 * i,
                                ap=[[WD, 128], [1, 1152]]),
                    in_=eq_stage)
                lk = k8t[X][pb:pb + 32, :, i * 128:(i + 1) * 128]
                bA = band_ps.tile([128, 1024], F32, name="bA", tag="bA")
                for wo in (0, 512):
                    nc.tensor.matmul(
                        bA[:, wo:wo + 512], lk,
                        de8p_sb[pb:pb + 32, :, base + wo:base + wo + 512],
                        start=True, stop=True, perf_mode=DR)
                bB = tail_ps.tile([128, 128], F32, name="bB", tag="bB")
                nc.tensor.matmul(
                    bB, lk,
                    de8p_sb[pb:pb + 32, :, base + 1024:base + 1152],
                    start=True, stop=True, perf_mode=DR)
                ek_stage = band_sb.tile([128, 1152], FP8, name="ek_stage", tag="ek_stage")
                nc.vector.tensor_scalar_mul(ek_stage[:, 0:1024], bA, 1.0 / 64.0)
                nc.vector.tensor_scalar_mul(ek_stage[:, 1024:1152], bB, 1.0 / 64.0)
                nc.sync.dma_start(
                    out=bass.AP(tensor=ekd[h],
                                offset=128 * i * WD + 896 - 128 * i,
                                ap=[[WD, 128], [1, 1152]]),
                    in_=ek_stage)

        # ---- phase A: projections interleaved with band tables ----
        proj_qk(0)
        proj_qk(1)
        bands(0)
        bands(1)
        proj_qk(2)
        bands(2)
        bands(3)
        proj_qk(3)
        bands(4)
        bands(5)
        for st in range(8):
            proj_v(st)
        bands(6)
        bands(7)

        # ---- phase B: per-head scores + PV ----
        # relQT[lt][p, r] = Eq[128*lt+p, 1023 + r - (128*lt+p)]  (plain
        # strided read, 2KB descriptors); rel_q is injected into the scores
        # PSUM by PE identity-matmuls (transposed accumulate).  relK8 is the
        # plain fp8 Ek read; a DVE scalar_tensor_tensor adds it and writes
        # the f32 exp input to SBUF.
        def rel_dmas(h):
            relq, relk = [], []
            for t in range(8):
                rq = relq_pool.tile([128, S], FP8, name="rq", tag="rq")
                nc.sync.dma_start(
                    out=rq,
                    in_=bass.AP(tensor=eqr[h],
                                offset=128 * t * (WD - 1) + 1023,
                                ap=[[WD - 1, 128], [1, 1024]]))
                relq.append(rq)
                rk = relk_pool.tile([128, S], FP8, name="rk", tag="rk")
                nc.gpsimd.dma_start(
                    out=rk,
                    in_=bass.AP(tensor=ekd[h],
                                offset=(WD - 1) * 128 * t + 1023,
                                ap=[[WD - 1, 128], [1, 1024]]))
                relk.append(rk)
            return relq, relk

        rel_q = [rel_dmas(0), rel_dmas(1)]
        for h in range(NHC):
            et, po = h // 2, 64 * (h % 2)
            relq, relk = rel_q.pop(0)
            if h + 2 < NHC:
                rel_q.append(rel_dmas(h + 2))
            expt = []
            for rt in range(8):
                r0 = rt * 128
                ex = expt_pool.tile([128, S], BF16, name="ex", tag="ex")
                for nh in range(2):
                    sch = score_ps.tile([128, 512], F32, name="sc", tag="sc")
                    nc.tensor.matmul(
                        sch,
                        kT_sb[et][po:po + 64, r0:r0 + 128],
                        qT_sb[et][po:po + 64, nh * 512:(nh + 1) * 512],
                        start=True, stop=True)
                    for j in range(4):
                        lt = nh * 4 + j
                        nc.tensor.matmul(
                            sch[:, j * 128:(j + 1) * 128],
                            relq[lt][:, r0:r0 + 128], id_sb,
                            start=False, stop=True,
                            skip_group_check=True)
                    exin = exin_pool.tile([128, 512], F32, name="exin", tag="exin")
                    nc.vector.scalar_tensor_tensor(
                        out=exin, in0=relk[rt][:, nh * 512:(nh + 1) * 512],
                        scalar=1.0, in1=sch, op0=ALU.bypass, op1=ALU.add)
                    nc.scalar.activation(out=ex[:, nh * 512:(nh + 1) * 512],
                                         in_=exin,
                                         func=ACT.Exp,
                                         scale=1.0 / math.sqrt(HD),
                                         bias=mcol_sb[:, rt:rt + 1])
                expt.append(ex)

            for lt in range(8):
                cxt = score_ps.tile([128, 512], F32, name="cxt", tag="sc")
                cx = cxt[:, 0:65]
                for rt in range(8):
                    nc.tensor.matmul(cx, expt[rt][:, lt * 128:(lt + 1) * 128],
                                     v_sb[rt][:, h, :],
                                     start=(rt == 0), stop=(rt == 7))
                rc = small.tile([128, 1], F32, name="rc", tag="rc")
                nc.vector.reciprocal(rc, cx[:, 64:65])
                nc.scalar.activation(out=out_sb[:, lt, h * 64:h * 64 + 64],
                                     in_=cx[:, 0:64],
                                     func=ACT.Copy,
                                     scale=rc[:, 0:1])
            # stream this head's output columns out as soon as PV finishes
            nc.scalar.dma_start(
                out=bass.AP(tensor=out_t, offset=h * 64,
                            ap=[[E, 128], [E * 128, 8], [1, 64]]),
                in_=out_sb[:, :, h * 64:h * 64 + 64])
    nc.compile()
    return nc


_NC_CACHE = {}
LAST_RESULT = None
LAST_IN_MAPS = None


def kernel(hidden_states, attention_mask, Wq, bq, Wk, bk, Wv, bv, dist_emb):
    hidden_states = np.asarray(hidden_states, np.float32)
    attention_mask = np.asarray(attention_mask, np.float32)
    Wq, bq = np.asarray(Wq, np.float32), np.asarray(bq, np.float32)
    Wk, bk = np.asarray(Wk, np.float32), np.asarray(bk, np.float32)
    Wv, bv = np.asarray(Wv, np.float32), np.asarray(bv, np.float32)
    dist_emb = np.asarray(dist_emb, np.float32)
    bf = mybir.dt.np(BF16)
    f8 = mybir.dt.np(FP8)

    deT = 64.0 * dist_emb.T  # [64, 2047], x64 into fp8 normal range
    de8P = np.zeros((128, 2, WD), np.float32)
    de8R = np.zeros((128, 2, WD), np.float32)
    for b in range(3):
        for kt in range(2):
            de8P[32 * b:32 * b + 32, kt, :2047] = deT[32 * kt:32 * kt + 32, :]
            de8R[32 * b:32 * b + 32, kt, :2047] = deT[32 * kt:32 * kt + 32, ::-1]
    de8P = de8P.reshape(128, 2 * WD)
    de8R = de8R.reshape(128, 2 * WD)

    id8v = np.zeros((64, 2, 128), np.float32)
    for kt in range(2):
        for p in range(64):
            id8v[p, kt, 64 * kt + p] = 1.0
    id8v = id8v.reshape(64, 256).astype(f8)

    if "nc" not in _NC_CACHE:
        _NC_CACHE["nc"] = build_nc()
    nc = _NC_CACHE["nc"]

    in_maps = []
    for c in range(8):
        b, g = c // 2, c % 2
        esl = slice(g * E, (g + 1) * E)
        m = attention_mask[b, 0, 0, :].astype(np.float32)
        in_maps.append({
            "hT": np.ascontiguousarray(hidden_states[b].T).astype(bf),
            "wqT": np.ascontiguousarray(Wq[esl, :].T).astype(bf),
            "wkT": np.ascontiguousarray(Wk[esl, :].T).astype(bf),
            "wvT": np.ascontiguousarray(Wv[esl, :].T).astype(bf),
            "bqr": np.ascontiguousarray(bq[esl][None, :]).astype(bf),
            "bkr": np.ascontiguousarray(bk[esl][None, :]).astype(bf),
            "bvr": np.ascontiguousarray(bv[esl][None, :]).astype(bf),
            "de8R": de8R.astype(f8), "de8P": de8P.astype(f8),
            "mcol": np.ascontiguousarray(m.reshape(8, 128).T),
            "ident": np.eye(128, dtype=np.float32).astype(f8),
        })
    import os as _os
    global LAST_RESULT, LAST_IN_MAPS
    LAST_IN_MAPS = in_maps
    res = run_bass_kernel_spmd(nc, in_maps, core_ids=list(range(8)),
                               trace=bool(_os.environ.get("KTRACE")),
                               tmpdir=_os.environ.get("KTRACE_DIR") or None)
    LAST_RESULT = res
    out = np.empty((B, S, D), np.float32)
    for c in range(8):
        b, g = c // 2, c % 2
        out[b, :, g * E:(g + 1) * E] = res.results[c]["out"]
    return out


# revision 17
# speedup vs baseline: 1.2625x; 1.2625x over previous
"""BitSelfAttention (relative_key_query position bias) on 8 trn2 cores.

Sharding: core c -> batch b=c//2, head-group g=c%2 (8 heads of 64 dims).
Per core: q/k/v projections for its 512 output dims, then per-head
attention with the Toeplitz relative-position bias realized via a DRAM
round-trip (skewed access patterns) for the Eq/Ek tables.

v3 structure:
 - Eq/Ek band-table matmuls run in fp8e4 DoubleRow perf mode (2x PE
   throughput).  q/k fp8 operands are built by gpsimd casting DMAs into
   [32, 2(ktile), S] layouts; the dist table is pre-scaled x64 host-side
   (fp8 dynamic range) and the evacuation copy divides by 64.
 - Band sections are EMITTED between projection blocks so the PE stream
   stays dense through the unthrottled startup window (the tensor engine
   executes its queue in program order).
 - Band PSUM is one [128,1152] tile (3 banks) so each table evacuates in
   a single engine op (eq -> scalar Copy/64 -> bf16, ek -> vector mul/64
   -> fp8).
 - No identity-matmul rel injection: a DVE scalar_tensor_tensor adds rel
   to the scores PSUM and writes bf16 to SBUF, freeing the PSUM bank
   before the exp; the attention mask rides the exp as a per-partition
   bias AP.
"""
import math
from contextlib import ExitStack

import numpy as np

import concourse.bass as bass
import concourse.bacc as bacc
import concourse.tile as tile
from concourse import mybir
from concourse.bass_utils import run_bass_kernel_spmd

B, S, D, H = 4, 1024, 1024, 16
HD = 64
E = 512          # output dims per core (8 heads)
NHC = 8          # heads per core
WD = 2176        # scratch DRAM row width (896 + 1280)
F32 = mybir.dt.float32
BF16 = mybir.dt.bfloat16
FP8 = mybir.dt.float8e4
DR = mybir.MatmulPerfMode.DoubleRow
ALU = mybir.AluOpType
ACT = mybir.ActivationFunctionType


def build_nc():
    nc = bacc.Bacc()
    hT = nc.declare_dram_parameter("hT", [D, S], BF16, isOutput=False)
    wqT = nc.declare_dram_parameter("wqT", [D, E], BF16, isOutput=False)
    wkT = nc.declare_dram_parameter("wkT", [D, E], BF16, isOutput=False)
    wvT = nc.declare_dram_parameter("wvT", [D, E], BF16, isOutput=False)
    bqr = nc.declare_dram_parameter("bqr", [1, E], BF16, isOutput=False)
    bkr = nc.declare_dram_parameter("bkr", [1, E], BF16, isOutput=False)
    bvr = nc.declare_dram_parameter("bvr", [1, E], BF16, isOutput=False)
    # fp8 dist tables, x64-scaled, [32, 2(ktile), WD] replicated on the
    # 0/32/64 partition blocks so the q8/k8 lhsT base always matches.
    de8R = nc.declare_dram_parameter("de8R", [128, 2 * WD], FP8, isOutput=False)
    de8P = nc.declare_dram_parameter("de8P", [128, 2 * WD], FP8, isOutput=False)
    mcol = nc.declare_dram_parameter("mcol", [128, 8], F32, isOutput=False)
    ident = nc.declare_dram_parameter("ident", [128, 128], FP8, isOutput=False)
    out_t = nc.declare_dram_parameter("out", [S, E], F32, isOutput=True)

    # DRAM scratch, fresh per head (no WAR fan-in on reuse)
    eqr = [nc.dram_tensor(f"eqr{i}", [S, WD], FP8) for i in range(8)]
    ekd = [nc.dram_tensor(f"ekd{i}", [S, WD], FP8) for i in range(8)]

    ctx = ExitStack()
    with ctx:
        tc = ctx.enter_context(tile.TileContext(nc))
        consts = ctx.enter_context(tc.tile_pool(name="consts", bufs=1))
        # PSUM (8 banks): band [128,1024] 2bk x2 + tail 1bk + scores/proj/ctx 1bk x3
        band_ps = ctx.enter_context(tc.tile_pool(name="band_ps", bufs=2, space="PSUM"))
        tail_ps = ctx.enter_context(tc.tile_pool(name="tail_ps", bufs=1, space="PSUM"))
        score_ps = ctx.enter_context(tc.tile_pool(name="score_ps", bufs=3, space="PSUM"))
        band_sb = ctx.enter_context(tc.tile_pool(name="band_sb", bufs=8))
        relq_pool = ctx.enter_context(tc.tile_pool(name="relq_pool", bufs=24))
        relk_pool = ctx.enter_context(tc.tile_pool(name="relk_pool", bufs=24))
        expt_pool = ctx.enter_context(tc.tile_pool(name="expt_pool", bufs=10))
        exin_pool = ctx.enter_context(tc.tile_pool(name="exin_pool", bufs=3))
        small = ctx.enter_context(tc.tile_pool(name="small", bufs=4))

        # ---- load inputs to SBUF (one big DMA per tensor) ----
        ht_big = consts.tile([128, 8, S], BF16, name="ht_big")
        nc.sync.dma_start(
            out=ht_big[:, 0:4, :],
            in_=bass.AP(tensor=hT, offset=0,
                        ap=[[S, 128], [128 * S, 4], [1, S]]))
        nc.scalar.dma_start(
            out=ht_big[:, 4:8, :],
            in_=bass.AP(tensor=hT, offset=4 * 128 * S,
                        ap=[[S, 128], [128 * S, 4], [1, S]]))
        ht_sb = [ht_big[:, kt, :] for kt in range(8)]
        wq_sb, wk_sb, wv_sb = [], [], []
        for (dst, src, nm, eng) in ((wq_sb, wqT, "wq", nc.sync),
                                    (wk_sb, wkT, "wk", nc.scalar),
                                    (wv_sb, wvT, "wv", nc.sync)):
            big = consts.tile([128, 8, E], BF16, name=f"{nm}_big")
            eng.dma_start(
                out=big,
                in_=bass.AP(tensor=src, offset=0,
                            ap=[[E, 128], [128 * E, 8], [1, E]]))
            for kt in range(8):
                dst.append(big[:, kt, :])
        de8r_sb = consts.tile([128, 2, WD], FP8, name="de8r_sb")
        nc.scalar.dma_start(out=de8r_sb,
                          in_=bass.AP(tensor=de8R, offset=0,
                                      ap=[[2 * WD, 128], [WD, 2], [1, WD]]))
        de8p_sb = consts.tile([128, 2, WD], FP8, name="de8p_sb")
        nc.scalar.dma_start(out=de8p_sb,
                          in_=bass.AP(tensor=de8P, offset=0,
                                      ap=[[2 * WD, 128], [WD, 2], [1, WD]]))
        mcol_sb = consts.tile([128, 8], F32, name="mcol_sb")
        nc.scalar.dma_start(out=mcol_sb, in_=mcol[:, :])
        id_sb = consts.tile([128, 128], FP8, name="id_sb")
        nc.scalar.dma_start(out=id_sb, in_=ident[:, :])
        br_sb = {}
        for nm, src in (("bq", bqr), ("bk", bkr), ("bv", bvr)):
            t = consts.tile([1, E], BF16, name=f"{nm}_sb")
            nc.scalar.dma_start(out=t, in_=src[:, :])
            br_sb[nm] = t
        ones_sb = consts.tile([1, E], BF16, name="ones_sb")
        nc.vector.memset(ones_sb, 1.0)

        qT_sb = [consts.tile([128, S], BF16, name=f"qT{et}") for et in range(4)]
        kT_sb = [consts.tile([128, S], BF16, name=f"kT{et}") for et in range(4)]
        # fp8 [32,2,S] operands, 3 heads per tile (base partition 0/32/64)
        q8t = [consts.tile([96, 2, S], FP8, name=f"q8{x}") for x in "ABC"]
        k8t = [consts.tile([96, 2, S], FP8, name=f"k8{x}") for x in "ABC"]
        v_sb = [consts.tile([128, 8, 65], BF16, name=f"v{st}") for st in range(8)]
        out_sb = consts.tile([128, 8, E], F32, name="out_sb")

        def proj_qk(et):
            for (w_sb, bias, dstl) in ((wq_sb, "bq", qT_sb), (wk_sb, "bk", kT_sb)):
                for ns in range(2):
                    ps = score_ps.tile([128, 512], F32, name="ps_proj", tag="sc")
                    for kt in range(8):
                        nc.tensor.matmul(
                            ps, w_sb[kt][:, et * 128:(et + 1) * 128],
                            ht_sb[kt][:, ns * 512:(ns + 1) * 512],
                            start=(kt == 0), stop=False)
                    nc.tensor.matmul(
                        ps, br_sb[bias][0:1, et * 128:(et + 1) * 128],
                        ones_sb[0:1, 0:512], start=False, stop=True)
                    if ns == 0:
                        nc.vector.tensor_copy(dstl[et][:, 0:512], ps)
                    else:
                        nc.scalar.copy(dstl[et][:, 512:1024], ps)
            # fp8 prep for heads 2*et, 2*et+1 (gpsimd casting DMAs)
            for g in range(2):
                h = 2 * et + g
                X, pb = h // 3, 32 * (h % 3)
                for kt in range(2):
                    nc.gpsimd.dma_start(
                        out=q8t[X][pb:pb + 32, kt, :],
                        in_=qT_sb[et][64 * g + 32 * kt:64 * g + 32 * kt + 32, :])
                    nc.gpsimd.dma_start(
                        out=k8t[X][pb:pb + 32, kt, :],
                        in_=kT_sb[et][64 * g + 32 * kt:64 * g + 32 * kt + 32, :])

        def proj_v(st):
            ps = score_ps.tile([128, 512], F32, name="ps_proj", tag="sc")
            for kt in range(8):
                nc.tensor.matmul(
                    ps, ht_sb[kt][:, st * 128:(st + 1) * 128],
                    wv_sb[kt], start=(kt == 0), stop=False)
            nc.tensor.matmul(ps, ones_sb[0:1, 0:128], br_sb["bv"],
                             start=False, stop=True)
            nc.vector.tensor_copy(v_sb[st][:, :, 0:64],
                                  ps.rearrange("p (h e) -> p h e", h=8))
            nc.vector.memset(v_sb[st][:, :, 64:65], 1.0)

        def bands(h):
            X, pb = h // 3, 32 * (h % 3)
            for i in range(8):
                base = 896 - 128 * i
                lq = q8t[X][pb:pb + 32, :, i * 128:(i + 1) * 128]
                bA = band_ps.tile([128, 1024], F32, name="bA", tag="bA")
                for wo in (0, 512):
                    nc.tensor.matmul(
                        bA[:, wo:wo + 512], lq,
                        de8r_sb[pb:pb + 32, :, base + wo:base + wo + 512],
                        start=True, stop=True, perf_mode=DR)
                bB = tail_ps.tile([128, 128], F32, name="bB", tag="bB")
                nc.tensor.matmul(
                    bB, lq,
                    de8r_sb[pb:pb + 32, :, base + 1024:base + 1152],
                    start=True, stop=True, perf_mode=DR)
                eq_stage = band_sb.tile([128, 1152], FP8, name="eq_stage", tag="eq_stage")
                nc.scalar.activation(out=eq_stage[:, 0:1024], in_=bA,
                                     func=ACT.Copy, scale=1.0 / 64.0)
                nc.scalar.activation(out=eq_stage[:, 1024:1152], in_=bB,
                                     func=ACT.Copy, scale=1.0 / 64.0)
                nc.sync.dma_start(
                    out=bass.AP(tensor=eqr[h],
                                offset=128 * i * WD + 896 - 128 * i,
                                ap=[[WD, 128], [1, 1152]]),
                    in_=eq_stage)
                lk = k8t[X][pb:pb + 32, :, i * 128:(i + 1) * 128]
                bA = band_ps.tile([128, 1024], F32, name="bA", tag="bA")
                for wo in (0, 512):
                    nc.tensor.matmul(
                        bA[:, wo:wo + 512], lk,
                        de8p_sb[pb:pb + 32, :, base + wo:base + wo + 512],
                        start=True, stop=True, perf_mode=DR)
                bB = tail_ps.tile([128, 128], F32, name="bB", tag="bB")
                nc.tensor.matmul(
                    bB, lk,
                    de8p_sb[pb:pb + 32, :, base + 1024:base + 1152],
                    start=True, stop=True, perf_mode=DR)
                ek_stage = band_sb.tile([128, 1152], FP8, name="ek_stage", tag="ek_stage")
                nc.vector.tensor_scalar_mul(ek_stage[:, 0:1024], bA, 1.0 / 64.0)
                nc.vector.tensor_scalar_mul(ek_stage[:, 1024:1152], bB, 1.0 / 64.0)
                nc.sync.dma_start(
                    out=bass.AP(tensor=ekd[h],
                                offset=128 * i * WD + 896 - 128 * i,
                                ap=[[WD, 128], [1, 1152]]),
                    in_=ek_stage)

        # ---- phase A/B interleaved schedule ----
        # relQT[lt][p, r] = Eq[128*lt+p, 1023 + r - (128*lt+p)]  (plain
        # strided fp8 read); rel_q is injected into the scores PSUM by PE
        # identity-matmuls (transposed accumulate).  relK8 is the plain fp8
        # Ek read; a DVE scalar_tensor_tensor adds it and writes the f32
        # exp input to SBUF.
        def rel_dmas(h):
            relq, relk = [], []
            for t in range(8):
                rq = relq_pool.tile([128, S], FP8, name="rq", tag="rq")
                nc.sync.dma_start(
                    out=rq,
                    in_=bass.AP(tensor=eqr[h],
                                offset=128 * t * (WD - 1) + 1023,
                                ap=[[WD - 1, 128], [1, 1024]]))
                relq.append(rq)
                rk = relk_pool.tile([128, S], FP8, name="rk", tag="rk")
                nc.gpsimd.dma_start(
                    out=rk,
                    in_=bass.AP(tensor=ekd[h],
                                offset=(WD - 1) * 128 * t + 1023,
                                ap=[[WD - 1, 128], [1, 1024]]))
                relk.append(rk)
            return relq, relk

        def head_attn(h, rels):
            et, po = h // 2, 64 * (h % 2)
            relq, relk = rels
            expt = []
            for rt in range(8):
                r0 = rt * 128
                ex = expt_pool.tile([128, S], BF16, name="ex", tag="ex")
                for nh in range(2):
                    sch = score_ps.tile([128, 512], F32, name="sc", tag="sc")
                    nc.tensor.matmul(
                        sch,
                        kT_sb[et][po:po + 64, r0:r0 + 128],
                        qT_sb[et][po:po + 64, nh * 512:(nh + 1) * 512],
                        start=True, stop=True)
                    for j in range(4):
                        lt = nh * 4 + j
                        nc.tensor.matmul(
                            sch[:, j * 128:(j + 1) * 128],
                            relq[lt][:, r0:r0 + 128], id_sb,
                            start=False, stop=True,
                            skip_group_check=True)
                    exin = exin_pool.tile([128, 512], F32, name="exin", tag="exin")
                    nc.vector.scalar_tensor_tensor(
                        out=exin, in0=relk[rt][:, nh * 512:(nh + 1) * 512],
                        scalar=1.0, in1=sch, op0=ALU.bypass, op1=ALU.add)
                    nc.scalar.activation(out=ex[:, nh * 512:(nh + 1) * 512],
                                         in_=exin,
                                         func=ACT.Exp,
                                         scale=1.0 / math.sqrt(HD),
                                         bias=mcol_sb[:, rt:rt + 1])
                expt.append(ex)

            for lt in range(8):
                cxt = score_ps.tile([128, 512], F32, name="cxt", tag="sc")
                cx = cxt[:, 0:65]
                for rt in range(8):
                    nc.tensor.matmul(cx, expt[rt][:, lt * 128:(lt + 1) * 128],
                                     v_sb[rt][:, h, :],
                                     start=(rt == 0), stop=(rt == 7))
                rc = small.tile([128, 1], F32, name="rc", tag="rc")
                nc.vector.reciprocal(rc, cx[:, 64:65])
                nc.scalar.activation(out=out_sb[:, lt, h * 64:h * 64 + 64],
                                     in_=cx[:, 0:64],
                                     func=ACT.Copy,
                                     scale=rc[:, 0:1])
            # stream this head's output columns out as soon as PV finishes
            nc.scalar.dma_start(
                out=bass.AP(tensor=out_t, offset=h * 64,
                            ap=[[E, 128], [E * 128, 8], [1, 64]]),
                in_=out_sb[:, :, h * 64:h * 64 + 64])

        R = {}
        proj_qk(0)
        proj_qk(1)
        bands(0)
        bands(1)
        proj_qk(2)
        bands(2)
        R[0] = rel_dmas(0)
        bands(3)
        R[1] = rel_dmas(1)
        proj_qk(3)
        bands(4)
        R[2] = rel_dmas(2)
        bands(5)
        for st in range(8):
            proj_v(st)
        bands(6)
        head_attn(0, R.pop(0))
        R[3] = rel_dmas(3)
        bands(7)
        head_attn(1, R.pop(1))
        R[4] = rel_dmas(4)
        for h in range(2, NHC):
            head_attn(h, R.pop(h))
            if h + 3 < NHC:
                R[h + 3] = rel_dmas(h + 3)
    nc.compile()
    return nc


_NC_CACHE = {}
LAST_RESULT = None
LAST_IN_MAPS = None


def kernel(hidden_states, attention_mask, Wq, bq, Wk, bk, Wv, bv, dist_emb):
    hidden_states = np.asarray(hidden_states, np.float32)
    attention_mask = np.asarray(attention_mask, np.float32)
    Wq, bq = np.asarray(Wq, np.float32), np.asarray(bq, np.float32)
    Wk, bk = np.asarray(Wk, np.float32), np.asarray(bk, np.float32)
    Wv, bv = np.asarray(Wv, np.float32), np.asarray(bv, np.float32)
    dist_emb = np.asarray(dist_emb, np.float32)
    bf = mybir.dt.np(BF16)
    f8 = mybir.dt.np(FP8)

    deT = 64.0 * dist_emb.T  # [64, 2047], x64 into fp8 normal range
    de8P = np.zeros((128, 2, WD), np.float32)
    de8R = np.zeros((128, 2, WD), np.float32)
    for b in range(3):
        for kt in range(2):
            de8P[32 * b:32 * b + 32, kt, :2047] = deT[32 * kt:32 * kt + 32, :]
            de8R[32 * b:32 * b + 32, kt, :2047] = deT[32 * kt:32 * kt + 32, ::-1]
    de8P = de8P.reshape(128, 2 * WD)
    de8R = de8R.reshape(128, 2 * WD)

    id8v = np.zeros((64, 2, 128), np.float32)
    for kt in range(2):
        for p in range(64):
            id8v[p, kt, 64 * kt + p] = 1.0
    id8v = id8v.reshape(64, 256).astype(f8)

    if "nc" not in _NC_CACHE:
        _NC_CACHE["nc"] = build_nc()
    nc = _NC_CACHE["nc"]

    in_maps = []
    for c in range(8):
        b, g = c // 2, c % 2
        esl = slice(g * E, (g + 1) * E)
        m = attention_mask[b, 0, 0, :].astype(np.float32)
        in_maps.append({
            "hT": np.ascontiguousarray(hidden_states[b].T).astype(bf),
            "wqT": np.ascontiguousarray(Wq[esl, :].T).astype(bf),
            "wkT": np.ascontiguousarray(Wk[esl, :].T).astype(bf),
            "wvT": np.ascontiguousarray(Wv[esl, :].T).astype(bf),
            "bqr": np.ascontiguousarray(bq[esl][None, :]).astype(bf),
            "bkr": np.ascontiguousarray(bk[esl][None, :]).astype(bf),
            "bvr": np.ascontiguousarray(bv[esl][None, :]).astype(bf),
            "de8R": de8R.astype(f8), "de8P": de8P.astype(f8),
            "mcol": np.ascontiguousarray(m.reshape(8, 128).T),
            "ident": np.eye(128, dtype=np.float32).astype(f8),
        })
    import os as _os
    global LAST_RESULT, LAST_IN_MAPS
    LAST_IN_MAPS = in_maps
    res = run_bass_kernel_spmd(nc, in_maps, core_ids=list(range(8)),
                               trace=bool(_os.environ.get("KTRACE")),
                               tmpdir=_os.environ.get("KTRACE_DIR") or None)
    LAST_RESULT = res
    out = np.empty((B, S, D), np.float32)
    for c in range(8):
        b, g = c // 2, c % 2
        out[b, :, g * E:(g + 1) * E] = res.results[c]["out"]
    return out


# revision 20
# speedup vs baseline: 3.7520x; 2.9719x over previous
"""BitSelfAttention (relative_key_query position bias) on 8 trn2 cores.

Sharding: core c -> batch b=c//2, head-group g=c%2 (8 heads of 64 dims).
Per core: q/k/v projections for its 512 output dims, then per-head
attention with the Toeplitz relative-position bias realized via a DRAM
round-trip (skewed access patterns) for the Eq/Ek tables.

v3 structure:
 - Eq/Ek band-table matmuls run in fp8e4 DoubleRow perf mode (2x PE
   throughput).  q/k fp8 operands are built by gpsimd casting DMAs into
   [32, 2(ktile), S] layouts; the dist table is pre-scaled x64 host-side
   (fp8 dynamic range) and the evacuation copy divides by 64.
 - Band sections are EMITTED between projection blocks so the PE stream
   stays dense through the unthrottled startup window (the tensor engine
   executes its queue in program order).
 - Band PSUM is one [128,1152] tile (3 banks) so each table evacuates in
   a single engine op (eq -> scalar Copy/64 -> bf16, ek -> vector mul/64
   -> fp8).
 - No identity-matmul rel injection: a DVE scalar_tensor_tensor adds rel
   to the scores PSUM and writes bf16 to SBUF, freeing the PSUM bank
   before the exp; the attention mask rides the exp as a per-partition
   bias AP.
"""
import math
from contextlib import ExitStack

import numpy as np

import concourse.bass as bass
import concourse.bacc as bacc
import concourse.tile as tile
from concourse import mybir
from concourse.bass_utils import run_bass_kernel_spmd

B, S, D, H = 4, 1024, 1024, 16
HD = 64
E = 512          # output dims per core (8 heads)
NHC = 8          # heads per core
WD = 2176        # scratch DRAM row width (896 + 1280)
F32 = mybir.dt.float32
BF16 = mybir.dt.bfloat16
FP8 = mybir.dt.float8e4
DR = mybir.MatmulPerfMode.DoubleRow
ALU = mybir.AluOpType
ACT = mybir.ActivationFunctionType


def build_nc():
    nc = bacc.Bacc()
    hT = nc.declare_dram_parameter("hT", [D, S], BF16, isOutput=False)
    wqT = nc.declare_dram_parameter("wqT", [D, E], BF16, isOutput=False)
    wkT = nc.declare_dram_parameter("wkT", [D, E], BF16, isOutput=False)
    wvT = nc.declare_dram_parameter("wvT", [D, E], BF16, isOutput=False)
    bqr = nc.declare_dram_parameter("bqr", [1, E], BF16, isOutput=False)
    bkr = nc.declare_dram_parameter("bkr", [1, E], BF16, isOutput=False)
    bvr = nc.declare_dram_parameter("bvr", [1, E], BF16, isOutput=False)
    # fp8 dist tables, x64-scaled, [32, 2(ktile), WD] replicated on the
    # 0/32/64 partition blocks so the q8/k8 lhsT base always matches.
    de8R = nc.declare_dram_parameter("de8R", [128, 2 * WD], FP8, isOutput=False)
    de8P = nc.declare_dram_parameter("de8P", [128, 2 * WD], FP8, isOutput=False)
    mcol = nc.declare_dram_parameter("mcol", [128, 8], F32, isOutput=False)
    ident = nc.declare_dram_parameter("ident", [128, 128], FP8, isOutput=False)
    out_t = nc.declare_dram_parameter("out", [S, E], F32, isOutput=True)

    # DRAM scratch, fresh per head (no WAR fan-in on reuse)
    eqr = [nc.dram_tensor(f"eqr{i}", [S, WD], FP8) for i in range(8)]
    ekd = [nc.dram_tensor(f"ekd{i}", [S, WD], FP8) for i in range(8)]

    ctx = ExitStack()
    with ctx:
        tc = ctx.enter_context(tile.TileContext(nc))
        consts = ctx.enter_context(tc.tile_pool(name="consts", bufs=1))
        # PSUM (8 banks): band [128,1024] 2bk x2 + tail 1bk + scores/proj/ctx 1bk x3
        band_ps = ctx.enter_context(tc.tile_pool(name="band_ps", bufs=2, space="PSUM"))
        tail_ps = ctx.enter_context(tc.tile_pool(name="tail_ps", bufs=1, space="PSUM"))
        score_ps = ctx.enter_context(tc.tile_pool(name="score_ps", bufs=3, space="PSUM"))
        band_sb = ctx.enter_context(tc.tile_pool(name="band_sb", bufs=8))
        relq_pool = ctx.enter_context(tc.tile_pool(name="relq_pool", bufs=24))
        relk_pool = ctx.enter_context(tc.tile_pool(name="relk_pool", bufs=24))
        expt_pool = ctx.enter_context(tc.tile_pool(name="expt_pool", bufs=10))
        exin_pool = ctx.enter_context(tc.tile_pool(name="exin_pool", bufs=3))
        small = ctx.enter_context(tc.tile_pool(name="small", bufs=4))

        # ---- load inputs to SBUF (one big DMA per tensor) ----
        ht_big = consts.tile([128, 8, S], BF16, name="ht_big")
        nc.sync.dma_start(
            out=ht_big[:, 0:4, :],
            in_=bass.AP(tensor=hT, offset=0,
                        ap=[[S, 128], [128 * S, 4], [1, S]]))
        nc.scalar.dma_start(
            out=ht_big[:, 4:8, :],
            in_=bass.AP(tensor=hT, offset=4 * 128 * S,
                        ap=[[S, 128], [128 * S, 4], [1, S]]))
        ht_sb = [ht_big[:, kt, :] for kt in range(8)]
        wq_sb, wk_sb, wv_sb = [], [], []
        for (dst, src, nm, eng) in ((wq_sb, wqT, "wq", nc.sync),
                                    (wk_sb, wkT, "wk", nc.scalar),
                                    (wv_sb, wvT, "wv", nc.sync)):
            big = consts.tile([128, 8, E], BF16, name=f"{nm}_big")
            eng.dma_start(
                out=big,
                in_=bass.AP(tensor=src, offset=0,
                            ap=[[E, 128], [128 * E, 8], [1, E]]))
            for kt in range(8):
                dst.append(big[:, kt, :])
        de8r_sb = consts.tile([128, 2, WD], FP8, name="de8r_sb")
        nc.scalar.dma_start(out=de8r_sb,
                          in_=bass.AP(tensor=de8R, offset=0,
                                      ap=[[2 * WD, 128], [WD, 2], [1, WD]]))
        de8p_sb = consts.tile([128, 2, WD], FP8, name="de8p_sb")
        nc.scalar.dma_start(out=de8p_sb,
                          in_=bass.AP(tensor=de8P, offset=0,
                                      ap=[[2 * WD, 128], [WD, 2], [1, WD]]))
        mcol_sb = consts.tile([128, 8], F32, name="mcol_sb")
        nc.scalar.dma_start(out=mcol_sb, in_=mcol[:, :])
        id_sb = consts.tile([128, 128], FP8, name="id_sb")
        nc.scalar.dma_start(out=id_sb, in_=ident[:, :])
        br_sb = {}
        for nm, src in (("bq", bqr), ("bk", bkr), ("bv", bvr)):
            t = consts.tile([1, E], BF16, name=f"{nm}_sb")
            nc.scalar.dma_start(out=t, in_=src[:, :])
            br_sb[nm] = t
        ones_sb = consts.tile([1, E], BF16, name="ones_sb")
        nc.vector.memset(ones_sb, 1.0)

        qT_sb = [consts.tile([128, S], BF16, name=f"qT{et}") for et in range(4)]
        kT_sb = [consts.tile([128, S], BF16, name=f"kT{et}") for et in range(4)]
        # fp8 [32,2,S] operands, 3 heads per tile (base partition 0/32/64)
        q8t = [consts.tile([96, 2, S], FP8, name=f"q8{x}") for x in "ABC"]
        k8t = [consts.tile([96, 2, S], FP8, name=f"k8{x}") for x in "ABC"]
        v_sb = [consts.tile([128, 8, 65], BF16, name=f"v{st}") for st in range(8)]
        out_sb = consts.tile([128, 8, E], F32, name="out_sb")

        def proj_qk(et):
            for (w_sb, bias, dstl) in ((wq_sb, "bq", qT_sb), (wk_sb, "bk", kT_sb)):
                for ns in range(2):
                    ps = score_ps.tile([128, 512], F32, name="ps_proj", tag="sc")
                    for kt in range(8):
                        nc.tensor.matmul(
                            ps, w_sb[kt][:, et * 128:(et + 1) * 128],
                            ht_sb[kt][:, ns * 512:(ns + 1) * 512],
                            start=(kt == 0), stop=False)
                    nc.tensor.matmul(
                        ps, br_sb[bias][0:1, et * 128:(et + 1) * 128],
                        ones_sb[0:1, 0:512], start=False, stop=True)
                    if ns == 0:
                        nc.vector.tensor_copy(dstl[et][:, 0:512], ps)
                    else:
                        nc.scalar.copy(dstl[et][:, 512:1024], ps)
            # fp8 prep for heads 2*et, 2*et+1 (gpsimd casting DMAs)
            for g in range(2):
                h = 2 * et + g
                X, pb = h // 3, 32 * (h % 3)
                for kt in range(2):
                    nc.gpsimd.dma_start(
                        out=q8t[X][pb:pb + 32, kt, :],
                        in_=qT_sb[et][64 * g + 32 * kt:64 * g + 32 * kt + 32, :])
                    nc.gpsimd.dma_start(
                        out=k8t[X][pb:pb + 32, kt, :],
                        in_=kT_sb[et][64 * g + 32 * kt:64 * g + 32 * kt + 32, :])

        def proj_v(st):
            ps = score_ps.tile([128, 512], F32, name="ps_proj", tag="sc")
            for kt in range(8):
                nc.tensor.matmul(
                    ps, ht_sb[kt][:, st * 128:(st + 1) * 128],
                    wv_sb[kt], start=(kt == 0), stop=False)
            nc.tensor.matmul(ps, ones_sb[0:1, 0:128], br_sb["bv"],
                             start=False, stop=True)
            nc.vector.tensor_copy(v_sb[st][:, :, 0:64],
                                  ps.rearrange("p (h e) -> p h e", h=8))
            nc.vector.memset(v_sb[st][:, :, 64:65], 1.0)

        def bands(h):
            X, pb = h // 3, 32 * (h % 3)
            for i in range(8):
                base = 896 - 128 * i
                lq = q8t[X][pb:pb + 32, :, i * 128:(i + 1) * 128]
                bA = band_ps.tile([128, 1024], F32, name="bA", tag="bA")
                for wo in (0, 512):
                    nc.tensor.matmul(
                        bA[:, wo:wo + 512], lq,
                        de8r_sb[pb:pb + 32, :, base + wo:base + wo + 512],
                        start=True, stop=True, perf_mode=DR)
                bB = tail_ps.tile([128, 128], F32, name="bB", tag="bB")
                nc.tensor.matmul(
                    bB, lq,
                    de8r_sb[pb:pb + 32, :, base + 1024:base + 1152],
                    start=True, stop=True, perf_mode=DR)
                eq_stage = band_sb.tile([128, 1152], FP8, name="eq_stage", tag="eq_stage")
                nc.scalar.activation(out=eq_stage[:, 0:1024], in_=bA,
                                     func=ACT.Copy, scale=1.0 / 64.0)
                nc.scalar.activation(out=eq_stage[:, 1024:1152], in_=bB,
                                     func=ACT.Copy, scale=1.0 / 64.0)
                nc.sync.dma_start(
                    out=bass.AP(tensor=eqr[h],
                                offset=128 * i * WD + 896 - 128 * i,
                                ap=[[WD, 128], [1, 1152]]),
                    in_=eq_stage)
                lk = k8t[X][pb:pb + 32, :, i * 128:(i + 1) * 128]
                bA = band_ps.tile([128, 1024], F32, name="bA", tag="bA")
                for wo in (0, 512):
                    nc.tensor.matmul(
                        bA[:, wo:wo + 512], lk,
                        de8p_sb[pb:pb + 32, :, base + wo:base + wo + 512],
                        start=True, stop=True, perf_mode=DR)
                bB = tail_ps.tile([128, 128], F32, name="bB", tag="bB")
                nc.tensor.matmul(
                    bB, lk,
                    de8p_sb[pb:pb + 32, :, base + 1024:base + 1152],
                    start=True, stop=True, perf_mode=DR)
                ek_stage = band_sb.tile([128, 1152], FP8, name="ek_stage", tag="ek_stage")
                nc.vector.tensor_scalar_mul(ek_stage[:, 0:1024], bA, 1.0 / 64.0)
                nc.vector.tensor_scalar_mul(ek_stage[:, 1024:1152], bB, 1.0 / 64.0)
                nc.sync.dma_start(
                    out=bass.AP(tensor=ekd[h],
                                offset=128 * i * WD + 896 - 128 * i,
                                ap=[[WD, 128], [1, 1152]]),
                    in_=ek_stage)

        # ---- phase A/B interleaved schedule ----
        # relQT[lt][p, r] = Eq[128*lt+p, 1023 + r - (128*lt+p)]  (plain
        # strided fp8 read); rel_q is injected into the scores PSUM by PE
        # identity-matmuls (transposed accumulate).  relK8 is the plain fp8
        # Ek read; a DVE scalar_tensor_tensor adds it and writes the f32
        # exp input to SBUF.
        def rel_dmas(h):
            relq, relk = [], []
            for t in range(8):
                rq = relq_pool.tile([128, S], FP8, name="rq", tag="rq")
                nc.sync.dma_start(
                    out=rq,
                    in_=bass.AP(tensor=eqr[h],
                                offset=128 * t * (WD - 1) + 1023,
                                ap=[[WD - 1, 128], [1, 1024]]))
                relq.append(rq)
                rk = relk_pool.tile([128, S], FP8, name="rk", tag="rk")
                nc.gpsimd.dma_start(
                    out=rk,
                    in_=bass.AP(tensor=ekd[h],
                                offset=(WD - 1) * 128 * t + 1023,
                                ap=[[WD - 1, 128], [1, 1024]]))
                relk.append(rk)
            return relq, relk

        def head_attn(h, rels):
            et, po = h // 2, 64 * (h % 2)
            relq, relk = rels
            expt = []
            for rt in range(8):
                r0 = rt * 128
                ex = expt_pool.tile([128, S], BF16, name="ex", tag="ex")
                for nh in range(2):
                    sch = score_ps.tile([128, 512], F32, name="sc", tag="sc")
                    nc.tensor.matmul(
                        sch,
                        kT_sb[et][po:po + 64, r0:r0 + 128],
                        qT_sb[et][po:po + 64, nh * 512:(nh + 1) * 512],
                        start=True, stop=True)
                    for j in range(4):
                        lt = nh * 4 + j
                        nc.tensor.matmul(
                            sch[:, j * 128:(j + 1) * 128],
                            relq[lt][:, r0:r0 + 128], id_sb,
                            start=False, stop=True,
                            skip_group_check=True)
                    exin = exin_pool.tile([128, 512], F32, name="exin", tag="exin")
                    nc.vector.scalar_tensor_tensor(
                        out=exin, in0=relk[rt][:, nh * 512:(nh + 1) * 512],
                        scalar=1.0, in1=sch, op0=ALU.bypass, op1=ALU.add)
                    nc.scalar.activation(out=ex[:, nh * 512:(nh + 1) * 512],
                                         in_=exin,
                                         func=ACT.Exp,
                                         scale=1.0 / math.sqrt(HD),
                                         bias=mcol_sb[:, rt:rt + 1])
                expt.append(ex)

            for lt in range(8):
                cxt = score_ps.tile([128, 512], F32, name="cxt", tag="sc")
                cx = cxt[:, 0:65]
                for rt in range(8):
                    nc.tensor.matmul(cx, expt[rt][:, lt * 128:(lt + 1) * 128],
                                     v_sb[rt][:, h, :],
                                     start=(rt == 0), stop=(rt == 7))
                rc = small.tile([128, 1], F32, name="rc", tag="rc")
                nc.vector.reciprocal(rc, cx[:, 64:65])
                nc.scalar.activation(out=out_sb[:, lt, h * 64:h * 64 + 64],
                                     in_=cx[:, 0:64],
                                     func=ACT.Copy,
                                     scale=rc[:, 0:1])
            # stream this head's output columns out as soon as PV finishes
            nc.scalar.dma_start(
                out=bass.AP(tensor=out_t, offset=h * 64,
                            ap=[[E, 128], [E * 128, 8], [1, 64]]),
                in_=out_sb[:, :, h * 64:h * 64 + 64])

        R = {}
        proj_qk(0)
        proj_qk(1)
        bands(0)
        bands(1)
        proj_qk(2)
        bands(2)
        R[0] = rel_dmas(0)
        bands(3)
        R[1] = rel_dmas(1)
        proj_qk(3)
        bands(4)
        R[2] = rel_dmas(2)
        bands(5)
        for st in range(8):
            proj_v(st)
        bands(6)
        head_attn(0, R.pop(0))
        R[3] = rel_dmas(3)
        bands(7)
        head_attn(1, R.pop(1))
        R[4] = rel_dmas(4)
        for h in range(2, NHC):
            head_attn(h, R.pop(h))
            if h + 3 < NHC:
                R[h + 3] = rel_dmas(h + 3)
    nc.compile()
    return nc


_NC_CACHE = {}
LAST_RESULT = None
LAST_IN_MAPS = None


def kernel(hidden_states, attention_mask, Wq, bq, Wk, bk, Wv, bv, dist_emb):
    hidden_states = np.asarray(hidden_states, np.float32)
    attention_mask = np.asarray(attention_mask, np.float32)
    Wq, bq = np.asarray(Wq, np.float32), np.asarray(bq, np.float32)
    Wk, bk = np.asarray(Wk, np.float32), np.asarray(bk, np.float32)
    Wv, bv = np.asarray(Wv, np.float32), np.asarray(bv, np.float32)
    dist_emb = np.asarray(dist_emb, np.float32)
    bf = mybir.dt.np(BF16)
    f8 = mybir.dt.np(FP8)

    deT = 64.0 * dist_emb.T  # [64, 2047], x64 into fp8 normal range
    de8P = np.zeros((128, 2, WD), np.float32)
    de8R = np.zeros((128, 2, WD), np.float32)
    for b in range(3):
        for kt in range(2):
            de8P[32 * b:32 * b + 32, kt, :2047] = deT[32 * kt:32 * kt + 32, :]
            de8R[32 * b:32 * b + 32, kt, :2047] = deT[32 * kt:32 * kt + 32, ::-1]
    de8P = de8P.reshape(128, 2 * WD)
    de8R = de8R.reshape(128, 2 * WD)

    id8v = np.zeros((64, 2, 128), np.float32)
    for kt in range(2):
        for p in range(64):
            id8v[p, kt, 64 * kt + p] = 1.0
    id8v = id8v.reshape(64, 256).astype(f8)

    if "nc" not in _NC_CACHE:
        _NC_CACHE["nc"] = build_nc()
    nc = _NC_CACHE["nc"]

    in_maps = []
    for c in range(8):
        b, g = c // 2, c % 2
        esl = slice(g * E, (g + 1) * E)
        m = attention_mask[b, 0, 0, :].astype(np.float32)
        in_maps.append({
            "hT": np.ascontiguousarray(hidden_states[b].T).astype(bf),
            "wqT": np.ascontiguousarray(Wq[esl, :].T).astype(bf),
            "wkT": np.ascontiguousarray(Wk[esl, :].T).astype(bf),
            "wvT": np.ascontiguousarray(Wv[esl, :].T).astype(bf),
            "bqr": np.ascontiguousarray(bq[esl][None, :]).astype(bf),
            "bkr": np.ascontiguousarray(bk[esl][None, :]).astype(bf),
            "bvr": np.ascontiguousarray(bv[esl][None, :]).astype(bf),
            "de8R": de8R.astype(f8), "de8P": de8P.astype(f8),
            "mcol": np.ascontiguousarray(m.reshape(8, 128).T),
            "ident": np.eye(128, dtype=np.float32).astype(f8),
        })
    import os as _os
    global LAST_RESULT, LAST_IN_MAPS
    LAST_IN_MAPS = in_maps
    res = run_bass_kernel_spmd(nc, in_maps, core_ids=list(range(8)),
                               trace=bool(_os.environ.get("KTRACE")),
                               tmpdir=_os.environ.get("KTRACE_DIR") or None)
    LAST_RESULT = res
    out = np.empty((B, S, D), np.float32)
    for c in range(8):
        b, g = c // 2, c % 2
        out[b, :, g * E:(g + 1) * E] = res.results[c]["out"]
    return out
